# revision 12
# baseline (speedup 1.0000x reference)
"""DerivativeNet (direction='x') on 8 Trainium2 NeuronCores.

Contract: kernel(u, mask) takes FULL inputs
  u    [16, 2, 1024, 1024] f32
  mask [16, 1, 1024, 1024] f32
returns FULL output [16, 2, 1024, 1024] f32.

Sharding: pure data-parallel over batch — 2 samples per core, 8 cores.

Per-row math along W (h = 0.01, zero-padded):
  d[k]   = up[k+1] - up[k]           (up = [0, u, 0])
  out[w] = p'[w]*d[w+1] + q'[w]*d[w]
  p' = eroded/(2h) + (cs==1)/h
  q' = eroded/(2h) + ((cs==total)&m)/h
  eroded = (cs[w+1]-cs[w-2] == 3),  cs = cumsum(m) along w

The mask pipeline runs in fp16 (values are small integers, exact in fp16
up to 2048 >= W). The u data path stays fp32.
"""

import sys

if "/opt/trn_rl_repo" not in sys.path:
    sys.path.insert(0, "/opt/trn_rl_repo")

import numpy as np

_B, _C, _H, _W = 16, 2, 1024, 1024
_NCORES = 8
_BS = _B // _NCORES              # batch per core
_INV_H = 100.0
_INV_2H = 50.0

# engine/tuning configuration
CFG = dict(
    S=2,                 # h-segments per SBUF tile
    bufs=3,              # tile pool buffers
    # u-side pass engines: "dve" or "gp"; fadd also "pe"
    dsub="gp",
    fadd="gp",
    # how many of the 16 u-tiles route dsub/fadd to gpsimd (rest dve)
    dsub_gp_frac=1.0,
    fadd_gp_frac=1.0,
    box="dve",
    pq="dve",            # pco-add/qco-mult/qco-add engine
    er_act=False,        # eroded via ACT relu instead of DVE ts
    pco_act=False,       # (cs==1) via ACT abs+relu instead of DVE ts
)

_CACHE = {}


def _build_nc(cfg=None):
    import concourse.tile as tile
    from concourse import bacc, mybir

    cfg = dict(CFG, **(cfg or {}))
    F32 = mybir.dt.float32
    F16 = mybir.dt.float16
    Alu = mybir.AluOpType

    nc = bacc.Bacc("TRN2", target_bir_lowering=False, debug=False,
                   enable_asserts=False, num_devices=_NCORES)
    u_ap = nc.dram_tensor("u", [_BS, _C, _H, _W], F32,
                          kind="ExternalInput").ap()
    m_ap = nc.dram_tensor("mask", [_BS, _H, _W], F32,
                          kind="ExternalInput").ap()
    o_ap = nc.dram_tensor("out", [_BS, _C, _H, _W], F32,
                          kind="ExternalOutput").ap()

    P, S, W = 128, cfg["S"], _W
    R = P * S
    HT = _H // R
    Wp = W + 4
    Wu = W + 2
    NU = _BS * HT * _C           # total u tiles

    def eng(name):
        return {"dve": nc.vector, "gp": nc.gpsimd}[cfg[name]]

    use_pe = cfg["fadd"] == "pe"

    with tile.TileContext(nc) as tc:
        with _stack() as ctx:
            pool = ctx.enter_context(tc.tile_pool(name="dn", bufs=cfg["bufs"]))
            cpool = ctx.enter_context(tc.tile_pool(name="cn", bufs=1))

            def const_col(val):
                t = cpool.tile([P, 1], F32, tag=f"c{val}")
                nc.gpsimd.memset(t[:], val)
                return t

            if cfg["er_act"]:
                bias_er = const_col(-2.0 * _INV_2H)
            if cfg["pco_act"]:
                bias_m1 = const_col(-1.0)
                bias_ph = const_col(_INV_H)
            if use_pe:
                ppool = ctx.enter_context(
                    tc.tile_pool(name="ps", bufs=2, space="PSUM"))
                ident = cpool.tile([P, P], F32, tag="ident")
                nc.gpsimd.memset(ident[:], 1.0)
                nc.gpsimd.affine_select(
                    ident[:], ident[:], [[-1, P]], Alu.is_equal, 0.0,
                    base=0, channel_multiplier=1)

            uidx = 0
            for b in range(_BS):
                for ht in range(HT):
                    r0 = ht * R
                    m32 = pool.tile([P, S, W], F32, tag="m32")
                    msrc = m_ap[b, r0:r0 + R, :].rearrange(
                        "(s p) w -> p s w", p=P)
                    nc.sync.dma_start(m32[:], msrc)
                    mf = pool.tile([P, S, W], F16, tag="mf")
                    nc.scalar.copy(mf[:], m32[:])

                    csp = pool.tile([P, S, Wp], F16, tag="csp")
                    nc.gpsimd.memset(csp[:, :, 0:2], 0.0)
                    for s in range(S):
                        nc.vector.tensor_tensor_scan(
                            csp[:, s, 2:2 + W], mf[:, s, :], mf[:, s, :],
                            0.0, Alu.add, Alu.bypass)
                    nc.scalar.copy(csp[:, :, 2 + W:3 + W],
                                   csp[:, :, 1 + W:2 + W])

                    cs = csp[:, :, 2:2 + W]
                    box = pool.tile([P, S, W], F16, tag="box")
                    eng("box").tensor_sub(box[:], csp[:, :, 3:3 + W],
                                          csp[:, :, 0:W])
                    er = pool.tile([P, S, W], F16, tag="er")
                    if cfg["er_act"]:
                        nc.scalar.activation(
                            er[:], box[:],
                            mybir.ActivationFunctionType.Relu,
                            bias=bias_er[:], scale=_INV_2H)
                    else:
                        nc.vector.tensor_scalar(er[:], box[:], 2.5, _INV_2H,
                                                Alu.is_ge, Alu.mult)
                    pco = pool.tile([P, S, W], F16, tag="pco")
                    if cfg["pco_act"]:
                        nc.scalar.activation(
                            pco[:], cs, mybir.ActivationFunctionType.Abs,
                            bias=bias_m1[:])
                        nc.scalar.activation(
                            pco[:], pco[:],
                            mybir.ActivationFunctionType.Relu,
                            bias=bias_ph[:], scale=-2.0 * _INV_H)
                    else:
                        nc.vector.tensor_scalar(pco[:], cs, 1.0, _INV_H,
                                                Alu.is_equal, Alu.mult)
                    eng("pq").tensor_add(pco[:], pco[:], er[:])
                    tot32 = pool.tile([P, S, 1], F32, tag="tot32")
                    nc.scalar.copy(tot32[:], csp[:, :, 1 + W:2 + W])
                    qco = pool.tile([P, S, W], F16, tag="qco")
                    for s in range(S):
                        nc.vector.tensor_scalar(
                            qco[:, s, :], csp[:, s, 2:2 + W],
                            tot32[:, s, :], _INV_H,
                            Alu.is_equal, Alu.mult)
                    eng("pq").tensor_mul(qco[:], qco[:], mf[:])
                    eng("pq").tensor_add(qco[:], qco[:], er[:])

                    for c in range(_C):
                        e_dsub = (nc.gpsimd if (cfg["dsub"] == "gp" and
                                  uidx < cfg["dsub_gp_frac"] * NU)
                                  else nc.vector)
                        up = pool.tile([P, S, Wu], F32, tag="up")
                        nc.gpsimd.memset(up[:, :, 0:1], 0.0)
                        nc.gpsimd.memset(up[:, :, W + 1:W + 2], 0.0)
                        usrc = u_ap[b, c, r0:r0 + R, :].rearrange(
                            "(s p) w -> p s w", p=P)
                        nc.sync.dma_start(up[:, :, 1:1 + W], usrc)
                        d = pool.tile([P, S, Wu], F32, tag="d")
                        e_dsub.tensor_sub(d[:, :, 0:W + 1],
                                          up[:, :, 1:W + 2],
                                          up[:, :, 0:W + 1])
                        t1 = pool.tile([P, S, W], F32, tag="t1")
                        nc.vector.tensor_mul(t1[:], pco[:],
                                             d[:, :, 1:1 + W])
                        nc.vector.tensor_mul(d[:, :, 0:W], qco[:],
                                             d[:, :, 0:W])
                        odst = o_ap[b, c, r0:r0 + R, :].rearrange(
                            "(s p) w -> p s w", p=P)
                        if use_pe:
                            ot = pool.tile([P, S, W], F32, tag="ot")
                            for s in range(S):
                                pt = ppool.tile([P, W], F32, tag="pt")
                                for j in range(0, W, 512):
                                    nc.tensor.matmul(
                                        pt[:, j:j + 512], ident[:],
                                        t1[:, s, j:j + 512],
                                        start=True, stop=False)
                                    nc.tensor.matmul(
                                        pt[:, j:j + 512], ident[:],
                                        d[:, s, j:j + 512],
                                        start=False, stop=True)
                                nc.scalar.copy(ot[:, s, :], pt[:])
                            nc.sync.dma_start(odst, ot[:])
                        else:
                            e_fadd = (nc.gpsimd if (cfg["fadd"] == "gp" and
                                      uidx < cfg["fadd_gp_frac"] * NU)
                                      else nc.vector)
                            e_fadd.tensor_add(t1[:], t1[:], d[:, :, 0:W])
                            nc.sync.dma_start(odst, t1[:])
                        uidx += 1
    nc.compile()
    return nc


def _stack():
    from contextlib import ExitStack
    return ExitStack()


def _get_runner():
    """Build, compile and jit once; return a callable
    (u_full, mask_full) -> out_full that just executes."""
    if "runner" in _CACHE:
        return _CACHE["runner"]

    import jax
    from jax.sharding import Mesh, PartitionSpec
    from jax.experimental.shard_map import shard_map
    from concourse import bass2jax, mybir

    nc = _build_nc()
    bass2jax.install_neuronx_cc_hook()

    partition_name = (nc.partition_id_tensor.name
                      if nc.partition_id_tensor else None)
    in_names = []
    out_names = []
    out_avals = []
    zero_shapes = []
    for alloc in nc.m.functions[0].allocations:
        if not isinstance(alloc, mybir.MemoryLocationSet):
            continue
        name = alloc.memorylocations[0].name
        if alloc.kind == "ExternalInput":
            if name != partition_name:
                in_names.append(name)
        elif alloc.kind == "ExternalOutput":
            out_names.append(name)
            shape = tuple(alloc.tensor_shape)
            dtype = mybir.dt.np(alloc.dtype)
            out_avals.append(jax.core.ShapedArray(shape, dtype))
            zero_shapes.append((shape, dtype))
    n_params = len(in_names)
    all_names = in_names + out_names
    if partition_name is not None:
        all_names = all_names + [partition_name]

    def _body(*args):
        operands = list(args)
        if partition_name is not None:
            operands.append(bass2jax.partition_id_tensor())
        outs = bass2jax._bass_exec_p.bind(
            *operands,
            out_avals=tuple(out_avals),
            in_names=tuple(all_names),
            out_names=tuple(out_names),
            lowering_input_output_aliases=(),
            sim_require_finite=True,
            sim_require_nnan=True,
            nc=nc,
        )
        return tuple(outs)

    devices = jax.devices()[:_NCORES]
    mesh = Mesh(np.asarray(devices), ("core",))
    n_outs = len(out_names)
    sharded = jax.jit(
        shard_map(_body, mesh=mesh,
                  in_specs=(PartitionSpec("core"),) * (n_params + n_outs),
                  out_specs=(PartitionSpec("core"),) * n_outs,
                  check_rep=False),
        donate_argnums=tuple(range(n_params, n_params + n_outs)),
        keep_unused=True,
    )

    name_to_idx = {n: i for i, n in enumerate(in_names)}

    def run(u_full, mask_full):
        u_full = np.ascontiguousarray(u_full, dtype=np.float32)
        mask_full = np.ascontiguousarray(
            mask_full, dtype=np.float32).reshape(_B, _H, _W)
        # per-core shard along axis 0 = declared per-core shape, so the
        # [16, ...] batch-major arrays are already the global view
        args = [None] * n_params
        args[name_to_idx["u"]] = u_full
        args[name_to_idx["mask"]] = mask_full
        zeros = [np.zeros((_NCORES * s[0], *s[1:]), d)
                 for (s, d) in zero_shapes]
        out_arrs = sharded(*args, *zeros)
        out = np.asarray(out_arrs[out_names.index("out")])
        return out.reshape(_B, _C, _H, _W)

    _CACHE["runner"] = run
    return run


def kernel(u, mask):
    run = _get_runner()
    return run(u, mask)


if __name__ == "__main__":
    rng = np.random.default_rng(0)
    u = rng.standard_normal((_B, _C, _H, _W), dtype=np.float32)
    mask = (rng.random((_B, 1, _H, _W)) < 0.5).astype(np.float32)
    out = kernel(u=u, mask=mask)
    print("out", out.shape, out.dtype, float(np.abs(out).max()))
